# revision 17
# baseline (speedup 1.0000x reference)
"""Trainium2 Bass kernel for nn_Beta_score2 (gnn_message_passing).

Computation (per batch element b):
  nodes   = 6 feature vectors x_k (padded to 2048; padding never contributes)
  temp_k  = tanh(x_k @ W[:, :d_k]^T + b)          # [512]
  score_k = temp_k . h_n                           # scalar
  beta    = softmax(score)                         # [6]
  z       = sum_k beta_k * pad(x_k)                # [2048], cols 1024: always 0

Sharding: data-parallel over batch, B=8192 -> 1024 per core on 8 cores.
On-chip layout is feature-major ("xT": features on partitions, batch on the
free dim) so the matmul contraction needs no on-chip transposes; x is
transposed on the host per shard.

Pipeline per core (per 512-wide batch chunk):
  stage 1: PE matmuls W^T-chunks x xT-chunks -> PSUM [128o, 512b];
           ACT fused bias+tanh -> temp^T in SBUF (f32)
  stage 2: PE matmuls h-columns (H6 trick) accumulate all 6 node scores into
           one PSUM [6, 512]; PE-transpose to batch-major, softmax on
           DVE/ACT (reduce_max -> exp(+bias, accum_out) -> recip -> mul);
           PE-transpose back, GPSIMD partition_broadcast -> beta tiles
  stage 3: DVE tensor-tensor weighted sum z^T = sum_k beta_k (.) x_k^T
           (cols 0-511: 6 terms, cols 512-1023: 2 terms), DMA out.
Output zT [1024, 1024] per core; host re-transposes and zero-pads to 2048.
"""

import os
import sys
import types

import numpy as np

B_TOTAL = 8192
NCORES = 8
BLOC = B_TOTAL // NCORES  # 1024
D = 4096                  # concatenated feature length
OUT = 512
DW = 1024                 # only W[:, :1024] is ever used
NODES = 6
NODE_OFF = [0, 1024, 1536, 2048, 2560, 3584]
NODE_DIM = [1024, 512, 512, 512, 1024, 512]
BC = 512                  # batch chunk on the free dim
NBC = BLOC // BC

MM_DTYPE = os.environ.get("KERNEL_MM_DTYPE", "float16")

LAST_EXEC_TIME_NS = None
LAST_RESULT = None

_cache = {}


def _install_ntff_hook():
    """run_bass_kernel_spmd(trace=True) under axon needs antenv.axon_hooks,
    which this image lacks; synthesize it from trn_agent_boot."""
    if "antenv.axon_hooks" in sys.modules:
        return
    try:
        import antenv
        import trn_agent_boot.trn_boot as tb
    except Exception:
        return
    mod = types.ModuleType("antenv.axon_hooks")
    _hook = tb._ntff_profile_via_ctypes("/opt/axon/libaxon_pjrt.so")
    mod.get_axon_ntff_profile_hook = lambda: _hook
    mod.set_axon_ntff_profile_hook = lambda h: None
    sys.modules["antenv.axon_hooks"] = mod
    antenv.axon_hooks = mod


def _build(mm_dtype_name):
    from contextlib import ExitStack

    import concourse.bacc as bacc
    import concourse.mybir as mybir
    import concourse.tile as tile

    f32 = mybir.dt.float32
    mm_dt = getattr(mybir.dt, mm_dtype_name)
    two_byte = mybir.dt.size(mm_dt) == 2
    s3_dt = mm_dt if two_byte else f32   # stage-3 elementwise dtype

    nc = bacc.Bacc("TRN2", target_bir_lowering=False, debug=False)
    xT_d = nc.dram_tensor("xT", [D, BLOC], mm_dt, kind="ExternalInput").ap()
    wT_d = nc.dram_tensor("wT", [DW, OUT], mm_dt, kind="ExternalInput").ap()
    bias_d = nc.dram_tensor("bias", [128, 4], f32, kind="ExternalInput").ap()
    h6_d = nc.dram_tensor("h6", [OUT, 36], mm_dt, kind="ExternalInput").ap()
    eye_d = nc.dram_tensor("eye", [128, 128], f32, kind="ExternalInput").ap()
    eyeh_d = nc.dram_tensor("eyeh", [128, 128], mm_dt, kind="ExternalInput").ap()
    zT_d = nc.dram_tensor("zT", [DW, BLOC], s3_dt, kind="ExternalOutput").ap()

    Tanh = mybir.ActivationFunctionType.Tanh
    Exp = mybir.ActivationFunctionType.Exp

    def rf(ap):
        # f32r tiles are viewed as plain f32 for vector-engine use;
        # 2-byte tiles are used natively (DVE 2x_1p mode)
        if two_byte or mm_dt == f32:
            return ap
        return ap.bitcast(f32)

    with tile.TileContext(nc) as tc, ExitStack() as ctx:
        const = ctx.enter_context(tc.tile_pool(name="const", bufs=1))
        wt_ocs = [const.tile([128, 8 * 128], mm_dt, name=f"wt_oc{o}") for o in range(4)]
        ones_h = const.tile([1, 128], mm_dt)
        bias_t = const.tile([128, 4], f32)
        h6_t = const.tile([128, 4 * 36], mm_dt)
        eye_t = const.tile([128, 128], f32)
        eyeh_t = const.tile([128, 128], mm_dt)

        # x^T resident as (4-kc group, bc) super-tiles: few DMA dispatches,
        # loaded in the order stage 1 consumes them (all of bc=0 first).
        GK = 4
        xgroups = {}

        def load_xgroup(g, b_):
            t = const.tile([128, GK, BC], mm_dt, name=f"xg_{g}_{b_}")
            nc.sync.dma_start(
                t[:],
                xT_d[:, b_ * BC : b_ * BC + BC].rearrange(
                    "(g p) b -> p g b", p=128
                )[:, g * GK : (g + 1) * GK, :],
            )
            xgroups[(g, b_)] = t

        nc.vector.memset(ones_h[:], 1.0)
        load_xgroup(2, 0)
        for o in range(4):
            nc.sync.dma_start(
                wt_ocs[o][:].rearrange("p (g w) -> p g w", g=8),
                wT_d[:, o * 128 : (o + 1) * 128].rearrange("(g p) w -> p g w", p=128),
            )
        nc.sync.dma_start(bias_t[:], bias_d[:, :])
        for oc in range(4):
            nc.sync.dma_start(
                h6_t[:, oc * 36 : (oc + 1) * 36], h6_d[oc * 128 : (oc + 1) * 128, :]
            )
        nc.sync.dma_start(eye_t[:], eye_d[:, :])
        nc.sync.dma_start(eyeh_t[:], eyeh_d[:, :])
        for g in (3, 4, 0, 1, 5, 6, 7):
            load_xgroup(g, 0)
        for g in (2, 3, 4, 0, 1, 5, 6, 7):
            load_xgroup(g, 1)

        pre_ps = ctx.enter_context(tc.tile_pool(name="pre", bufs=3, space="PSUM"))
        score_ps = ctx.enter_context(tc.tile_pool(name="score", bufs=2, space="PSUM"))
        tp_ps = ctx.enter_context(tc.tile_pool(name="tp", bufs=1, space="PSUM"))
        btp_ps = ctx.enter_context(tc.tile_pool(name="btp", bufs=2, space="PSUM"))
        temps = ctx.enter_context(tc.tile_pool(name="temps", bufs=4))
        small = ctx.enter_context(tc.tile_pool(name="small", bufs=2))
        bpool = ctx.enter_context(tc.tile_pool(name="bpool", bufs=2))
        zpool = ctx.enter_context(tc.tile_pool(name="zpool", bufs=3))

        def xts(kc, bc):
            return xgroups[(kc // GK, bc)][:, kc % GK, :]

        def wts(kc, oc):
            return wt_ocs[oc][:, kc * 128 : (kc + 1) * 128]

        for bc in range(NBC):
            # ---------- stage 1: temp^T tiles + score accumulation ----------
            sc = score_ps.tile([6, BC], f32)
            first_sc = True
            for n in (1, 2, 3, 0, 4, 5):
                nk = NODE_DIM[n] // 128
                off = NODE_OFF[n] // 128
                for oc in range(4):
                    ps = pre_ps.tile([128, BC], f32)
                    for kc in range(nk):
                        nc.tensor.matmul(
                            ps[:],
                            wts(kc, oc),
                            xts(off + kc, bc),
                            start=(kc == 0),
                            stop=(kc == nk - 1),
                        )
                    tt = temps.tile([128, BC], mm_dt)
                    nc.scalar.activation(
                        tt[:], ps[:], Tanh, bias=bias_t[:, oc : oc + 1], scale=1.0
                    )
                    nc.tensor.matmul(
                        sc[:],
                        h6_t[:, oc * 36 + 6 * n : oc * 36 + 6 * n + 6],
                        tt[:],
                        start=first_sc,
                        stop=(n == NODES - 1 and oc == 3),
                    )
                    first_sc = False

            # ---------- stage 2: softmax over the 6 nodes ----------
            sc_sb = small.tile([6, BC], f32, tag="sc_sb")
            nc.scalar.copy(sc_sb[:], sc[:])
            tp = tp_ps.tile([128, 24], f32)
            for j in range(4):
                nc.tensor.transpose(
                    tp[:, j * 6 : (j + 1) * 6],
                    sc_sb[:, j * 128 : (j + 1) * 128],
                    eye_t[0:6, 0:6],
                )
            expt = small.tile([128, 24], f32, tag="expt")
            sumexp = small.tile([128, 4], f32, tag="sumexp")
            nc.scalar.activation(expt[:], tp[:], Exp)
            nc.vector.tensor_reduce(
                sumexp[:],
                expt[:].rearrange("p (j k) -> p j k", j=4),
                axis=mybir.AxisListType.X,
                op=mybir.AluOpType.add,
            )
            rec = small.tile([128, 4], f32, tag="rec")
            nc.vector.reciprocal(rec[:], sumexp[:])
            beta = small.tile([128, 24], s3_dt, tag="beta")
            for j in range(4):
                nc.vector.tensor_scalar_mul(
                    beta[:, j * 6 : (j + 1) * 6],
                    expt[:, j * 6 : (j + 1) * 6],
                    rec[:, j : j + 1],
                )
            bts = [None] * NODES
            for k in (0, 4, 1, 2, 3, 5):
                btp = btp_ps.tile([1, BC], s3_dt, tag="btp")
                for j in range(4):
                    nc.tensor.transpose(
                        btp[0:1, j * 128 : (j + 1) * 128],
                        beta[:, j * 6 + k : j * 6 + k + 1],
                        eyeh_t[:] if two_byte else eye_t[:],
                    )
                stg = small.tile([1, BC], s3_dt, tag="stg")
                nc.scalar.copy(stg[:], btp[:])
                bt = bpool.tile([128, BC], s3_dt, tag=f"b{k}", name=f"bt{k}")
                nc.gpsimd.partition_broadcast(bt[:], stg[0:1, :])
                bts[k] = bt

            # ---------- stage 3: z^T = sum_k beta_k (.) x_k^T ----------
            for fc in (4, 5, 6, 7, 0, 1, 2, 3):
                if fc < 4:
                    contrib = [
                        (fc, 0),
                        (8 + fc, 1),
                        (12 + fc, 2),
                        (16 + fc, 3),
                        (20 + fc, 4),
                        (28 + fc, 5),
                    ]
                else:
                    contrib = [(fc, 0), (20 + fc, 4)]
                zt = zpool.tile([128, BC], s3_dt, tag="zt")
                i0, k0 = contrib[0]
                nc.vector.tensor_mul(zt[:], rf(xts(i0, bc)), bts[k0][:])
                for i, k in contrib[1:]:
                    tmp = zpool.tile([128, BC], s3_dt, tag="tmp")
                    nc.vector.tensor_mul(tmp[:], rf(xts(i, bc)), bts[k][:])
                    nc.vector.tensor_add(zt[:], zt[:], tmp[:])
                nc.sync.dma_start(
                    zT_d[fc * 128 : (fc + 1) * 128, bc * BC : bc * BC + BC], zt[:]
                )

    nc.compile()
    return nc


def _get_nc():
    key = MM_DTYPE
    if key not in _cache:
        _cache[key] = _build(key)
    return _cache[key]


def kernel(result_ls, result_A, result_lm, result_AT, result_ds, result_dm, W, b, h_n):
    global LAST_EXEC_TIME_NS
    _install_ntff_hook()
    from concourse.bass_utils import run_bass_kernel_spmd

    nc = _get_nc()

    x = np.concatenate(
        [
            np.asarray(t, dtype=np.float32).reshape(B_TOTAL, -1)
            for t in (result_ls, result_A, result_lm, result_AT, result_ds, result_dm)
        ],
        axis=1,
    )  # [8192, 4096]
    W = np.asarray(W, dtype=np.float32)
    b = np.asarray(b, dtype=np.float32)
    h_n = np.asarray(h_n, dtype=np.float32)

    import concourse.mybir as mybir
    mm_np = mybir.dt.np(getattr(mybir.dt, MM_DTYPE))
    wT = np.ascontiguousarray(W[:, :DW].T).astype(mm_np)   # [1024, 512]
    bias = np.ascontiguousarray(b.reshape(4, 128).T)       # [128, 4]
    h6 = np.zeros((OUT, 36), dtype=np.float32)
    for k in range(NODES):
        h6[:, 6 * k + k] = h_n[:, 0]
    h6 = h6.astype(mm_np)
    eye = np.eye(128, dtype=np.float32)
    eyeh = np.eye(128, dtype=mm_np)

    in_maps = []
    for c in range(NCORES):
        xc = x[c * BLOC : (c + 1) * BLOC]                  # [1024, 4096]
        in_maps.append(
            {
                "xT": np.ascontiguousarray(xc.T).astype(mm_np),  # [4096, 1024]
                "wT": wT,
                "bias": bias,
                "h6": h6,
                "eye": eye,
                "eyeh": eyeh,
            }
        )

    global LAST_RESULT
    res = run_bass_kernel_spmd(nc, in_maps, list(range(NCORES)))
    LAST_RESULT = res
    LAST_EXEC_TIME_NS = res.exec_time_ns

    out = np.zeros((B_TOTAL, 1, 2048), dtype=np.float32)
    for c in range(NCORES):
        zt = res.results[c]["zT"]                    # [1024, 1024] feature-major
        out[c * BLOC : (c + 1) * BLOC, 0, :DW] = zt.T.astype(np.float32)
    return out


# revision 19
# speedup vs baseline: 1.0430x; 1.0430x over previous
"""Trainium2 Bass kernel for nn_Beta_score2 (gnn_message_passing).

Computation (per batch element b):
  nodes   = 6 feature vectors x_k (padded to 2048; padding never contributes)
  temp_k  = tanh(x_k @ W[:, :d_k]^T + b)          # [512]
  score_k = temp_k . h_n                           # scalar
  beta    = softmax(score)                         # [6]
  z       = sum_k beta_k * pad(x_k)                # [2048], cols 1024: always 0

Sharding: data-parallel over batch, B=8192 -> 1024 per core on 8 cores.
On-chip layout is feature-major ("xT": features on partitions, batch on the
free dim) so the matmul contraction needs no on-chip transposes; x is
transposed on the host per shard.

Pipeline per core (per 512-wide batch chunk):
  stage 1: PE matmuls W^T-chunks x xT-chunks -> PSUM [128o, 512b];
           ACT fused bias+tanh -> temp^T in SBUF (f32)
  stage 2: PE matmuls h-columns (H6 trick) accumulate all 6 node scores into
           one PSUM [6, 512]; PE-transpose to batch-major, softmax on
           DVE/ACT (reduce_max -> exp(+bias, accum_out) -> recip -> mul);
           PE-transpose back, GPSIMD partition_broadcast -> beta tiles
  stage 3: DVE tensor-tensor weighted sum z^T = sum_k beta_k (.) x_k^T
           (cols 0-511: 6 terms, cols 512-1023: 2 terms), DMA out.
Output zT [1024, 1024] per core; host re-transposes and zero-pads to 2048.
"""

import os
import sys
import types

import numpy as np

B_TOTAL = 8192
NCORES = 8
BLOC = B_TOTAL // NCORES  # 1024
D = 4096                  # concatenated feature length
OUT = 512
DW = 1024                 # only W[:, :1024] is ever used
NODES = 6
NODE_OFF = [0, 1024, 1536, 2048, 2560, 3584]
NODE_DIM = [1024, 512, 512, 512, 1024, 512]
BC = 512                  # batch chunk on the free dim
NBC = BLOC // BC

MM_DTYPE = os.environ.get("KERNEL_MM_DTYPE", "float16")

LAST_EXEC_TIME_NS = None
LAST_RESULT = None

_cache = {}


def _install_ntff_hook():
    """run_bass_kernel_spmd(trace=True) under axon needs antenv.axon_hooks,
    which this image lacks; synthesize it from trn_agent_boot."""
    if "antenv.axon_hooks" in sys.modules:
        return
    try:
        import antenv
        import trn_agent_boot.trn_boot as tb
    except Exception:
        return
    mod = types.ModuleType("antenv.axon_hooks")
    _hook = tb._ntff_profile_via_ctypes("/opt/axon/libaxon_pjrt.so")
    mod.get_axon_ntff_profile_hook = lambda: _hook
    mod.set_axon_ntff_profile_hook = lambda h: None
    sys.modules["antenv.axon_hooks"] = mod
    antenv.axon_hooks = mod


def _build(mm_dtype_name):
    from contextlib import ExitStack

    import concourse.bacc as bacc
    import concourse.mybir as mybir
    import concourse.tile as tile

    f32 = mybir.dt.float32
    mm_dt = getattr(mybir.dt, mm_dtype_name)
    two_byte = mybir.dt.size(mm_dt) == 2
    s3_dt = mm_dt if two_byte else f32   # stage-3 elementwise dtype

    nc = bacc.Bacc("TRN2", target_bir_lowering=False, debug=False)
    xT_d = nc.dram_tensor("xT", [D, BLOC], mm_dt, kind="ExternalInput").ap()
    wT_d = nc.dram_tensor("wT", [DW, OUT], mm_dt, kind="ExternalInput").ap()
    bias_d = nc.dram_tensor("bias", [128, 4], f32, kind="ExternalInput").ap()
    h6_d = nc.dram_tensor("h6", [OUT, 36], mm_dt, kind="ExternalInput").ap()
    eye_d = nc.dram_tensor("eye", [128, 128], f32, kind="ExternalInput").ap()
    eyeh_d = nc.dram_tensor("eyeh", [128, 128], mm_dt, kind="ExternalInput").ap()
    zT_d = nc.dram_tensor("zT", [DW, BLOC], s3_dt, kind="ExternalOutput").ap()

    Tanh = mybir.ActivationFunctionType.Tanh
    Exp = mybir.ActivationFunctionType.Exp

    def rf(ap):
        # f32r tiles are viewed as plain f32 for vector-engine use;
        # 2-byte tiles are used natively (DVE 2x_1p mode)
        if two_byte or mm_dt == f32:
            return ap
        return ap.bitcast(f32)

    with tile.TileContext(nc) as tc, ExitStack() as ctx:
        const = ctx.enter_context(tc.tile_pool(name="const", bufs=1))
        wt_all = const.tile([128, 8 * OUT], mm_dt)
        ones_h = const.tile([1, 128], mm_dt)
        bias_t = const.tile([128, 4], f32)
        h6_t = const.tile([128, 4 * 36], mm_dt)
        eye_t = const.tile([128, 128], f32)
        eyeh_t = const.tile([128, 128], mm_dt)

        # x^T resident as (4-kc group, bc) super-tiles: few DMA dispatches,
        # loaded in the order stage 1 consumes them (all of bc=0 first).
        GK = 4
        xgroups = {}

        def load_xgroup(g, b_):
            t = const.tile([128, GK, BC], mm_dt, name=f"xg_{g}_{b_}")
            nc.sync.dma_start(
                t[:],
                xT_d[:, b_ * BC : b_ * BC + BC].rearrange(
                    "(g p) b -> p g b", p=128
                )[:, g * GK : (g + 1) * GK, :],
            )
            xgroups[(g, b_)] = t

        nc.vector.memset(ones_h[:], 1.0)
        load_xgroup(2, 0)
        nc.sync.dma_start(
            wt_all[:].rearrange("p (g o) -> p g o", g=8),
            wT_d[:, :].rearrange("(g p) o -> p g o", p=128),
        )
        nc.sync.dma_start(bias_t[:], bias_d[:, :])
        for oc in range(4):
            nc.sync.dma_start(
                h6_t[:, oc * 36 : (oc + 1) * 36], h6_d[oc * 128 : (oc + 1) * 128, :]
            )
        nc.sync.dma_start(eye_t[:], eye_d[:, :])
        nc.sync.dma_start(eyeh_t[:], eyeh_d[:, :])
        for g in (3, 4, 0, 1, 5, 6, 7):
            load_xgroup(g, 0)
        for g in (2, 3, 4, 0, 1, 5, 6, 7):
            load_xgroup(g, 1)

        pre_ps = ctx.enter_context(tc.tile_pool(name="pre", bufs=3, space="PSUM"))
        score_ps = ctx.enter_context(tc.tile_pool(name="score", bufs=2, space="PSUM"))
        tp_ps = ctx.enter_context(tc.tile_pool(name="tp", bufs=1, space="PSUM"))
        btp_ps = ctx.enter_context(tc.tile_pool(name="btp", bufs=2, space="PSUM"))
        temps = ctx.enter_context(tc.tile_pool(name="temps", bufs=4))
        small = ctx.enter_context(tc.tile_pool(name="small", bufs=2))
        bpool = ctx.enter_context(tc.tile_pool(name="bpool", bufs=2))
        zpool = ctx.enter_context(tc.tile_pool(name="zpool", bufs=3))

        def xts(kc, bc):
            return xgroups[(kc // GK, bc)][:, kc % GK, :]

        def wts(kc, oc):
            return wt_all[:, kc * OUT + oc * 128 : kc * OUT + (oc + 1) * 128]

        for bc in range(NBC):
            # ---------- stage 1: temp^T tiles + score accumulation ----------
            sc = score_ps.tile([6, BC], f32)
            first_sc = True
            for n in (1, 2, 3, 0, 4, 5):
                nk = NODE_DIM[n] // 128
                off = NODE_OFF[n] // 128
                for oc in range(4):
                    ps = pre_ps.tile([128, BC], f32)
                    for kc in range(nk):
                        nc.tensor.matmul(
                            ps[:],
                            wts(kc, oc),
                            xts(off + kc, bc),
                            start=(kc == 0),
                            stop=(kc == nk - 1),
                        )
                    tt = temps.tile([128, BC], mm_dt)
                    nc.scalar.activation(
                        tt[:], ps[:], Tanh, bias=bias_t[:, oc : oc + 1], scale=1.0
                    )
                    nc.tensor.matmul(
                        sc[:],
                        h6_t[:, oc * 36 + 6 * n : oc * 36 + 6 * n + 6],
                        tt[:],
                        start=first_sc,
                        stop=(n == NODES - 1 and oc == 3),
                    )
                    first_sc = False

            # ---------- stage 2: softmax over the 6 nodes ----------
            sc_sb = small.tile([6, BC], f32, tag="sc_sb")
            nc.scalar.copy(sc_sb[:], sc[:])
            tp = tp_ps.tile([128, 24], f32)
            for j in range(4):
                nc.tensor.transpose(
                    tp[:, j * 6 : (j + 1) * 6],
                    sc_sb[:, j * 128 : (j + 1) * 128],
                    eye_t[0:6, 0:6],
                )
            expt = small.tile([128, 24], f32, tag="expt")
            sumexp = small.tile([128, 4], f32, tag="sumexp")
            nc.scalar.activation(expt[:], tp[:], Exp)
            nc.vector.tensor_reduce(
                sumexp[:],
                expt[:].rearrange("p (j k) -> p j k", j=4),
                axis=mybir.AxisListType.X,
                op=mybir.AluOpType.add,
            )
            rec = small.tile([128, 4], f32, tag="rec")
            nc.vector.reciprocal(rec[:], sumexp[:])
            beta = small.tile([128, 24], s3_dt, tag="beta")
            for j in range(4):
                nc.vector.tensor_scalar_mul(
                    beta[:, j * 6 : (j + 1) * 6],
                    expt[:, j * 6 : (j + 1) * 6],
                    rec[:, j : j + 1],
                )
            bts = [None] * NODES
            for k in (0, 4, 1, 2, 3, 5):
                btp = btp_ps.tile([1, BC], s3_dt, tag="btp")
                for j in range(4):
                    nc.tensor.transpose(
                        btp[0:1, j * 128 : (j + 1) * 128],
                        beta[:, j * 6 + k : j * 6 + k + 1],
                        eyeh_t[:] if two_byte else eye_t[:],
                    )
                stg = small.tile([1, BC], s3_dt, tag="stg")
                nc.scalar.copy(stg[:], btp[:])
                bt = bpool.tile([128, BC], s3_dt, tag=f"b{k}", name=f"bt{k}")
                if two_byte and bc == NBC - 1:
                    bp = pre_ps.tile([128, BC], f32, tag="ps", name=f"bp{k}")
                    nc.tensor.matmul(bp[:], ones_h[:], stg[0:1, :], start=True, stop=True)
                    nc.scalar.copy(bt[:], bp[:])
                else:
                    nc.gpsimd.partition_broadcast(bt[:], stg[0:1, :])
                bts[k] = bt

            # ---------- stage 3: z^T = sum_k beta_k (.) x_k^T ----------
            for fc in (4, 5, 6, 7, 0, 1, 2, 3):
                if fc < 4:
                    contrib = [
                        (fc, 0),
                        (8 + fc, 1),
                        (12 + fc, 2),
                        (16 + fc, 3),
                        (20 + fc, 4),
                        (28 + fc, 5),
                    ]
                else:
                    contrib = [(fc, 0), (20 + fc, 4)]
                zt = zpool.tile([128, BC], s3_dt, tag="zt")
                i0, k0 = contrib[0]
                nc.vector.tensor_mul(zt[:], rf(xts(i0, bc)), bts[k0][:])
                for i, k in contrib[1:]:
                    tmp = zpool.tile([128, BC], s3_dt, tag="tmp")
                    nc.vector.tensor_mul(tmp[:], rf(xts(i, bc)), bts[k][:])
                    nc.vector.tensor_add(zt[:], zt[:], tmp[:])
                nc.sync.dma_start(
                    zT_d[fc * 128 : (fc + 1) * 128, bc * BC : bc * BC + BC], zt[:]
                )

    nc.compile()
    return nc


def _get_nc():
    key = MM_DTYPE
    if key not in _cache:
        _cache[key] = _build(key)
    return _cache[key]


def kernel(result_ls, result_A, result_lm, result_AT, result_ds, result_dm, W, b, h_n):
    global LAST_EXEC_TIME_NS
    _install_ntff_hook()
    from concourse.bass_utils import run_bass_kernel_spmd

    nc = _get_nc()

    x = np.concatenate(
        [
            np.asarray(t, dtype=np.float32).reshape(B_TOTAL, -1)
            for t in (result_ls, result_A, result_lm, result_AT, result_ds, result_dm)
        ],
        axis=1,
    )  # [8192, 4096]
    W = np.asarray(W, dtype=np.float32)
    b = np.asarray(b, dtype=np.float32)
    h_n = np.asarray(h_n, dtype=np.float32)

    import concourse.mybir as mybir
    mm_np = mybir.dt.np(getattr(mybir.dt, MM_DTYPE))
    wT = np.ascontiguousarray(W[:, :DW].T).astype(mm_np)   # [1024, 512]
    bias = np.ascontiguousarray(b.reshape(4, 128).T)       # [128, 4]
    h6 = np.zeros((OUT, 36), dtype=np.float32)
    for k in range(NODES):
        h6[:, 6 * k + k] = h_n[:, 0]
    h6 = h6.astype(mm_np)
    eye = np.eye(128, dtype=np.float32)
    eyeh = np.eye(128, dtype=mm_np)

    in_maps = []
    for c in range(NCORES):
        xc = x[c * BLOC : (c + 1) * BLOC]                  # [1024, 4096]
        in_maps.append(
            {
                "xT": np.ascontiguousarray(xc.T).astype(mm_np),  # [4096, 1024]
                "wT": wT,
                "bias": bias,
                "h6": h6,
                "eye": eye,
                "eyeh": eyeh,
            }
        )

    global LAST_RESULT
    res = run_bass_kernel_spmd(nc, in_maps, list(range(NCORES)))
    LAST_RESULT = res
    LAST_EXEC_TIME_NS = res.exec_time_ns

    out = np.zeros((B_TOTAL, 1, 2048), dtype=np.float32)
    for c in range(NCORES):
        zt = res.results[c]["zT"]                    # [1024, 1024] feature-major
        out[c * BLOC : (c + 1) * BLOC, 0, :DW] = zt.T.astype(np.float32)
    return out
